# revision 19
# baseline (speedup 1.0000x reference)
"""Causal self-attention (B=2, S=4096, D=768, H=12) on 8 Trainium2 NeuronCores.

Sharding: data + head parallel. Core c handles batch c//4 and the 3 heads
starting at (c%4)*3. Each core computes the qkv projection for its heads,
causal attention, and a partial output projection (its heads' rows of w_out);
the host sums the 4 partial outputs per batch.

Device design notes:
 - x arrives pre-transposed (xT [768, 4096]) so the contraction dim lands on
   SBUF partitions for every projection matmul.
 - q, k are produced transposed ([hd, S]); scores are computed transposed
   ([sk, sq]) so the PV matmul consumes exp(scores) directly as the moving
   operand; a ones-column appended to v yields softmax denominators for free
   in the same matmul.
 - QK/projections run in float32r (full-rate fp32 path at free-dim >= 256);
   the PV and output projections run in bf16 (probs/ctx quantization only).
 - exp runs on ScalarE with the 1/sqrt(hd) scale fused into the activation
   affine; no max subtraction (scores are O(5) here, exp is safe in fp32).
 - Causal masking: only the 4 diagonal-chunk patterns need masking, applied
   as a GPSIMD affine_select (predicated fill) on exp(scores).
 - Softmax division: reciprocal_approx_fast (~51 ULP) on the denominator row,
   GPSIMD partition-broadcast, one DVE multiply.
 - All persistent activations are split into per-512-chunk tiles so the Tile
   scheduler can overlap projection, attention, and output phases.
"""

import numpy as np

try:
    import concourse.bass as bass  # noqa: F401
except ImportError:
    import sys
    sys.path.insert(0, "/opt/trn_rl_repo")

import concourse.bass as bass
import concourse.tile as tile
from concourse import bacc, mybir
from concourse.bass_utils import run_bass_kernel_spmd

F32 = mybir.dt.float32
F32R = mybir.dt.float32r
BF16 = mybir.dt.bfloat16
N_CORES = 8
B, S, D, H, HD = 2, 4096, 768, 12, 64
HPC = 3            # heads per core
SC = 512           # sequence chunk (free dim of most matmuls)
NSC = S // SC      # 8
KC = 128           # contraction chunk
NKC = D // KC      # 6
NQB = S // SC      # query blocks of 512
CPB = SC // KC     # key chunks per query block (4)
VW = HPC * (HD + 1)  # 195 v columns per key-chunk: [64 v | 1] x 3 heads

_CACHE = {}


def _emit(nc, tc, ins, out_ap):
    xT, wqk, wv, wo = ins
    MM = nc.tensor.matmul
    GE = mybir.AluOpType.is_ge

    constp = tc.alloc_tile_pool(name="const", bufs=1)
    xtp = tc.alloc_tile_pool(name="xt", bufs=8)
    qTp = tc.alloc_tile_pool(name="qTp", bufs=8)
    kTp = tc.alloc_tile_pool(name="kTp", bufs=8)
    q2p = tc.alloc_tile_pool(name="q2p", bufs=8)
    vp = tc.alloc_tile_pool(name="vp", bufs=8)
    ctxp = tc.alloc_tile_pool(name="ctx", bufs=24)
    expp = tc.alloc_tile_pool(name="exp", bufs=3)
    smp = tc.alloc_tile_pool(name="sm", bufs=3)
    ostp = tc.alloc_tile_pool(name="ost", bufs=3)
    psA = tc.alloc_tile_pool(name="psA", bufs=2, space="PSUM")
    psB = tc.alloc_tile_pool(name="psB", bufs=2, space="PSUM")

    # ---- constants ----
    wqk_sb = constp.tile([128, NKC * 384], F32R, tag="wqk")
    for k in range(NKC):
        nc.sync.dma_start(wqk_sb[:, k * 384:(k + 1) * 384], wqk[k * 128:(k + 1) * 128, :])
    wv_sb = constp.tile([128, NKC * 256], F32R, tag="wv")
    for k in range(NKC):
        nc.sync.dma_start(wv_sb[:, k * 256:(k + 1) * 256], wv[k * 128:(k + 1) * 128, :])
    wo3_sb = constp.tile([64, HPC * 768], BF16, tag="wo3")
    for h in range(HPC):
        nc.sync.dma_start(wo3_sb[:, h * 768:(h + 1) * 768], wo[h * 64:(h + 1) * 64, :])

    # persistent activations, one tile per 512-wide sequence chunk.
    # matmul needs lhsT/rhs on the same partitions: head A uses rows 0:64 and
    # head B rows 64:128 of qT/kT (row-tiled concurrent matmuls); head C's
    # q^T and k^T sit side by side at rows 0:64 of q2k2 ([.,0:512]=q, 512:1024=k).
    qT = [qTp.tile([128, SC], F32R, tag="qT", name=f"qT{i}") for i in range(NSC)]
    kT = [kTp.tile([128, SC], F32R, tag="kT", name=f"kT{i}") for i in range(NSC)]
    q2k2 = [q2p.tile([64, 2 * SC], F32R, tag="qk2", name=f"q2k2_{i}") for i in range(NSC)]
    vt = [vp.tile([128, CPB * VW], BF16, tag="v", name=f"vt{i}") for i in range(NSC)]
    ctx = {h: [ctxp.tile([64, SC], BF16, tag="ctx", name=f"ctx{h}_{i}")
               for i in range(NQB)] for h in range(HPC)}

    ones_st = smp.tile([128, CPB], F32, tag="ones")
    nc.vector.memset(ones_st[:], 1.0)
    for i in range(NSC):
        v_r = vt[i][:].rearrange("p (c h e) -> p c h e", h=HPC, e=HD + 1)
        for h in range(HPC):
            nc.vector.tensor_copy(v_r[:, :, h, HD], ones_st[:])

    # ---- phase A: qkv projection (per 512-chunk sc) ----
    for sc in range(NSC):
        xts = []
        for k in range(NKC):
            xt = xtp.tile([128, SC], F32R, tag="xt", name=f"xt{k}_{sc}")
            nc.sync.dma_start(xt[:], xT[k * 128:(k + 1) * 128, sc * SC:(sc + 1) * SC])
            xts.append(xt)
        # q/k projection, transposed outputs; m0=[qA|qB], m1=[qC|kC], m2=[kA|kB]
        ps = psA.tile([128, 3 * SC], F32, tag="ps", name=f"psqk{sc}")
        for m in range(3):
            for k in range(NKC):
                MM(ps[:, m * SC:(m + 1) * SC],
                   wqk_sb[:, k * 384 + m * 128: k * 384 + (m + 1) * 128],
                   xts[k][:],
                   start=(k == 0), stop=(k == NKC - 1))
        nc.vector.tensor_copy(qT[sc][:], ps[:, 0:SC])
        nc.vector.tensor_copy(kT[sc][:], ps[:, 2 * SC:3 * SC])
        nc.vector.tensor_copy(q2k2[sc][:, 0:SC], ps[0:64, SC:2 * SC])
        # kC sits at psum rows 64:128 but must land at rows 0:64 -> partition
        # shift via SBUF staging + DMA (DVE lanes cannot cross partitions)
        qst = smp.tile([64, SC], F32R, tag="qst", name=f"qst{sc}")
        nc.vector.tensor_copy(qst[:], ps[64:128, SC:2 * SC])
        nc.sync.dma_start(q2k2[sc][:, SC:2 * SC], qst[:])
        # v projection, natural layout, 4 row chunks of 128
        v_r = vt[sc][:].rearrange("p (c h e) -> p c h e", h=HPC, e=HD + 1)
        for j in range(CPB):
            pv = psB.tile([128, 256], F32, tag="pb", name=f"psv{sc}_{j}")
            for k in range(NKC):
                MM(pv[:], xts[k][:, j * 128:(j + 1) * 128],
                   wv_sb[:, k * 256:(k + 1) * 256],
                   start=(k == 0), stop=(k == NKC - 1))
            nc.vector.tensor_copy(
                v_r[:, j, :, 0:HD],
                pv[:, 0:HPC * HD].rearrange("p (h e) -> p h e", e=HD))

    # ---- phase B: attention, pair (heads A,B) + single (head C) per block ----
    def attention_block(sqb, hlist):
        nch = (sqb + 1) * CPB
        cps = {h: psB.tile([128, SC], F32, tag="pb", name=f"cps{h}_{sqb}")
               for h in hlist}
        slots = [(h, ck) for ck in range(nch) for h in hlist]
        for g0 in range(0, len(slots), 3):
            grp = slots[g0:g0 + 3]
            sg = psA.tile([128, 3 * SC], F32, tag="ps", name=f"sg{sqb}_{g0}")
            eg = expp.tile([128, 3 * SC], BF16, tag="exp", name=f"eg{sqb}_{g0}")
            for si, (h, ck) in enumerate(grp):
                osl = slice(si * SC, (si + 1) * SC)
                if h < 2:
                    b0 = h * 64
                    MM(sg[:, osl],
                       kT[ck // CPB][b0:b0 + 64, (ck % CPB) * 128:(ck % CPB + 1) * 128],
                       qT[sqb][b0:b0 + 64, :],
                       start=True, stop=True)
                else:
                    MM(sg[:, osl],
                       q2k2[ck // CPB][:, SC + (ck % CPB) * 128: SC + (ck % CPB + 1) * 128],
                       q2k2[sqb][:, 0:SC],
                       start=True, stop=True)
            n = len(grp) * SC
            nc.scalar.activation(eg[:, 0:n], sg[:, 0:n],
                                 mybir.ActivationFunctionType.Exp, scale=0.125)
            for si, (h, ck) in enumerate(grp):
                osl = slice(si * SC, (si + 1) * SC)
                if ck >= nch - CPB:  # diagonal chunk: causal mask via fill
                    o = (ck - (nch - CPB)) * 128
                    # keep where sq_local >= sk_local + o: -part + free - o >= 0
                    nc.gpsimd.affine_select(
                        eg[:, osl], eg[:, osl], pattern=[[1, SC]],
                        compare_op=GE, fill=0.0, base=-o, channel_multiplier=-1)
                v_r = vt[ck // CPB][:].rearrange("p (c h e) -> p c h e", h=HPC, e=HD + 1)
                MM(cps[h][0:HD + 1, :],
                   v_r[:, ck % CPB, h, :],
                   eg[:, osl],
                   start=(ck == 0), stop=(ck == nch - 1))
        for h in hlist:
            # reciprocal_approx_fast is a bitwise-seed op and misreads PSUM;
            # bounce the denominator row through SBUF first
            dn = smp.tile([1, SC], F32, tag="dn", name=f"dn{h}_{sqb}")
            nc.vector.tensor_copy(dn[:], cps[h][HD:HD + 1, :])
            rec = smp.tile([1, SC], F32, tag="rec", name=f"rec{h}_{sqb}")
            nc.vector.reciprocal_approx_fast(rec[:], dn[:])
            bc = smp.tile([64, SC], F32, tag="bc", name=f"bc{h}_{sqb}")
            nc.gpsimd.partition_broadcast(bc[:], rec[:])
            nc.vector.tensor_mul(ctx[h][sqb][:], cps[h][0:HD, :], bc[:])

    for sqb in range(NQB):
        attention_block(sqb, (0, 1))
        attention_block(sqb, (2,))

    # ---- phase C: partial output projection (this core's head rows of w_out) ----
    for sc2 in range(S // 128):
        po = psA.tile([128, 1024], F32, tag="ps", name=f"po{sc2}")
        csl = slice((sc2 % CPB) * 128, (sc2 % CPB + 1) * 128)
        for noff, nsz in ((0, 512), (512, 256)):
            for hi in range(HPC):
                MM(po[:, noff:noff + nsz],
                   ctx[hi][sc2 // CPB][:, csl],
                   wo3_sb[:, hi * 768 + noff: hi * 768 + noff + nsz],
                   start=(hi == 0), stop=(hi == HPC - 1))
        ost = ostp.tile([128, 768], F32, tag="ost", name=f"ost{sc2}")
        nc.vector.tensor_copy(ost[:], po[:, 0:768])
        nc.sync.dma_start(out_ap[sc2 * 128:(sc2 + 1) * 128, :], ost[:])

    for p in (psB, psA, ostp, smp, expp, ctxp, vp, q2p, kTp, qTp, xtp, constp):
        p.release()


def _build():
    if "nc" in _CACHE:
        return _CACHE["nc"]
    nc = bacc.Bacc("TRN2", target_bir_lowering=False, debug=False, num_devices=N_CORES)
    xT = nc.dram_tensor("xT", [D, S], F32R, kind="ExternalInput").ap()
    wqk = nc.dram_tensor("wqk", [D, 384], F32R, kind="ExternalInput").ap()
    wv = nc.dram_tensor("wv", [D, 256], F32R, kind="ExternalInput").ap()
    wo = nc.dram_tensor("wo", [HPC * HD, D], BF16, kind="ExternalInput").ap()
    out = nc.dram_tensor("out", [S, D], F32, kind="ExternalOutput").ap()
    with tile.TileContext(nc) as tc:
        _emit(nc, tc, (xT, wqk, wv, wo), out)
    nc.compile()
    _CACHE["nc"] = nc
    return nc


def _in_maps(x, w_qkv, w_out):
    import ml_dtypes
    xTs = [np.ascontiguousarray(x[b].T) for b in range(B)]
    maps = []
    for c in range(N_CORES):
        b = c // 4
        h0 = (c % 4) * HPC
        cols = lambda base, h: w_qkv[:, base + (h0 + h) * HD: base + (h0 + h + 1) * HD]
        wqk = np.ascontiguousarray(np.concatenate(
            [cols(0, 0), cols(0, 1),            # m0: qA | qB
             cols(0, 2), cols(D, 2),            # m1: qC | kC
             cols(D, 0), cols(D, 1)], axis=1))  # m2: kA | kB
        wv = np.ascontiguousarray(np.concatenate(
            [cols(2 * D, 0), cols(2 * D, 1), cols(2 * D, 2),
             np.zeros((D, 64), np.float32)], axis=1))
        wo = np.ascontiguousarray(
            w_out[h0 * HD:(h0 + HPC) * HD, :]).astype(ml_dtypes.bfloat16)
        maps.append({"xT": xTs[b], "wqk": wqk, "wv": wv, "wo": wo})
    return maps


def run_sharded(x, w_qkv, w_out, **spmd_kwargs):
    nc = _build()
    res = run_bass_kernel_spmd(nc, _in_maps(x, w_qkv, w_out),
                               list(range(N_CORES)), **spmd_kwargs)
    outs = [res.results[c]["out"] for c in range(N_CORES)]
    y = np.empty((B, S, D), np.float32)
    for b in range(B):
        y[b] = outs[4 * b] + outs[4 * b + 1] + outs[4 * b + 2] + outs[4 * b + 3]
    return y, res


def kernel(x, w_qkv, w_out):
    x = np.asarray(x, dtype=np.float32)
    w_qkv = np.asarray(w_qkv, dtype=np.float32)
    w_out = np.asarray(w_out, dtype=np.float32)
    y, _ = run_sharded(x, w_qkv, w_out)
    return y


# revision 20
# speedup vs baseline: 1.0029x; 1.0029x over previous
"""Causal self-attention (B=2, S=4096, D=768, H=12) on 8 Trainium2 NeuronCores.

Sharding: data + head parallel. Core c handles batch c//4 and the 3 heads
starting at (c%4)*3. Each core computes the qkv projection for its heads,
causal attention, and a partial output projection (its heads' rows of w_out);
the host sums the 4 partial outputs per batch.

Device design notes:
 - x arrives pre-transposed (xT [768, 4096]) so the contraction dim lands on
   SBUF partitions for every projection matmul.
 - q, k are produced transposed ([hd, S]); scores are computed transposed
   ([sk, sq]) so the PV matmul consumes exp(scores) directly as the moving
   operand; a ones-column appended to v yields softmax denominators for free
   in the same matmul.
 - QK/projections run in float32r (full-rate fp32 path at free-dim >= 256);
   the PV and output projections run in bf16 (probs/ctx quantization only).
 - exp runs on ScalarE with the 1/sqrt(hd) scale fused into the activation
   affine; no max subtraction (scores are O(5) here, exp is safe in fp32).
 - Causal masking: only the 4 diagonal-chunk patterns need masking, applied
   as a GPSIMD affine_select (predicated fill) on exp(scores).
 - Softmax division: reciprocal_approx_fast (~51 ULP) on the denominator row,
   GPSIMD partition-broadcast, one DVE multiply.
 - All persistent activations are split into per-512-chunk tiles so the Tile
   scheduler can overlap projection, attention, and output phases.
"""

import numpy as np

try:
    import concourse.bass as bass  # noqa: F401
except ImportError:
    import sys
    sys.path.insert(0, "/opt/trn_rl_repo")

import concourse.bass as bass
import concourse.tile as tile
from concourse import bacc, mybir
from concourse.bass_utils import run_bass_kernel_spmd

F32 = mybir.dt.float32
F32R = mybir.dt.float32r
BF16 = mybir.dt.bfloat16
N_CORES = 8
B, S, D, H, HD = 2, 4096, 768, 12, 64
HPC = 3            # heads per core
SC = 512           # sequence chunk (free dim of most matmuls)
NSC = S // SC      # 8
KC = 128           # contraction chunk
NKC = D // KC      # 6
NQB = S // SC      # query blocks of 512
CPB = SC // KC     # key chunks per query block (4)
VW = HPC * (HD + 1)  # 195 v columns per key-chunk: [64 v | 1] x 3 heads

_CACHE = {}


def _emit(nc, tc, ins, out_ap):
    xT, wqk, wv, wo = ins
    MM = nc.tensor.matmul
    GE = mybir.AluOpType.is_ge

    constp = tc.alloc_tile_pool(name="const", bufs=1)
    xtp = tc.alloc_tile_pool(name="xt", bufs=8)
    qTp = tc.alloc_tile_pool(name="qTp", bufs=8)
    kTp = tc.alloc_tile_pool(name="kTp", bufs=8)
    q2p = tc.alloc_tile_pool(name="q2p", bufs=8)
    vp = tc.alloc_tile_pool(name="vp", bufs=8)
    ctxp = tc.alloc_tile_pool(name="ctx", bufs=24)
    expp = tc.alloc_tile_pool(name="exp", bufs=3)
    smp = tc.alloc_tile_pool(name="sm", bufs=3)
    ostp = tc.alloc_tile_pool(name="ost", bufs=3)
    psA = tc.alloc_tile_pool(name="psA", bufs=2, space="PSUM")
    psB = tc.alloc_tile_pool(name="psB", bufs=2, space="PSUM")

    # ---- constants ----
    wqk_sb = constp.tile([128, NKC * 384], F32R, tag="wqk")
    for k in range(NKC):
        nc.sync.dma_start(wqk_sb[:, k * 384:(k + 1) * 384], wqk[k * 128:(k + 1) * 128, :])
    wv_sb = constp.tile([128, NKC * 256], F32R, tag="wv")
    for k in range(NKC):
        nc.sync.dma_start(wv_sb[:, k * 256:(k + 1) * 256], wv[k * 128:(k + 1) * 128, :])
    wo3_sb = constp.tile([64, HPC * 768], BF16, tag="wo3")
    for h in range(HPC):
        nc.sync.dma_start(wo3_sb[:, h * 768:(h + 1) * 768], wo[h * 64:(h + 1) * 64, :])

    # persistent activations, one tile per 512-wide sequence chunk.
    # matmul needs lhsT/rhs on the same partitions: head A uses rows 0:64 and
    # head B rows 64:128 of qT/kT (row-tiled concurrent matmuls); head C's
    # q^T and k^T sit side by side at rows 0:64 of q2k2 ([.,0:512]=q, 512:1024=k).
    qT = [qTp.tile([128, SC], F32R, tag="qT", name=f"qT{i}") for i in range(NSC)]
    kT = [kTp.tile([128, SC], F32R, tag="kT", name=f"kT{i}") for i in range(NSC)]
    q2k2 = [q2p.tile([64, 2 * SC], F32R, tag="qk2", name=f"q2k2_{i}") for i in range(NSC)]
    vt = [vp.tile([128, CPB * VW], BF16, tag="v", name=f"vt{i}") for i in range(NSC)]
    ctx = {h: [ctxp.tile([64, SC], BF16, tag="ctx", name=f"ctx{h}_{i}")
               for i in range(NQB)] for h in range(HPC)}

    ones_st = smp.tile([128, CPB], F32, tag="ones")
    nc.vector.memset(ones_st[:], 1.0)
    for i in range(NSC):
        v_r = vt[i][:].rearrange("p (c h e) -> p c h e", h=HPC, e=HD + 1)
        for h in range(HPC):
            nc.vector.tensor_copy(v_r[:, :, h, HD], ones_st[:])

    # ---- qkv projection for one 512-chunk ----
    def project_chunk(sc):
        xts = []
        for k in range(NKC):
            xt = xtp.tile([128, SC], F32R, tag="xt", name=f"xt{k}_{sc}")
            nc.sync.dma_start(xt[:], xT[k * 128:(k + 1) * 128, sc * SC:(sc + 1) * SC])
            xts.append(xt)
        # q/k projection, transposed outputs; m0=[qA|qB], m1=[qC|kC], m2=[kA|kB]
        ps = psA.tile([128, 3 * SC], F32, tag="ps", name=f"psqk{sc}")
        for m in range(3):
            for k in range(NKC):
                MM(ps[:, m * SC:(m + 1) * SC],
                   wqk_sb[:, k * 384 + m * 128: k * 384 + (m + 1) * 128],
                   xts[k][:],
                   start=(k == 0), stop=(k == NKC - 1))
        nc.vector.tensor_copy(qT[sc][:], ps[:, 0:SC])
        nc.vector.tensor_copy(kT[sc][:], ps[:, 2 * SC:3 * SC])
        nc.vector.tensor_copy(q2k2[sc][:, 0:SC], ps[0:64, SC:2 * SC])
        # kC sits at psum rows 64:128 but must land at rows 0:64 -> partition
        # shift via SBUF staging + DMA (DVE lanes cannot cross partitions)
        qst = smp.tile([64, SC], F32R, tag="qst", name=f"qst{sc}")
        nc.vector.tensor_copy(qst[:], ps[64:128, SC:2 * SC])
        nc.sync.dma_start(q2k2[sc][:, SC:2 * SC], qst[:])
        # v projection, natural layout, 4 row chunks of 128
        v_r = vt[sc][:].rearrange("p (c h e) -> p c h e", h=HPC, e=HD + 1)
        for j in range(CPB):
            pv = psB.tile([128, 256], F32, tag="pb", name=f"psv{sc}_{j}")
            for k in range(NKC):
                MM(pv[:], xts[k][:, j * 128:(j + 1) * 128],
                   wv_sb[:, k * 256:(k + 1) * 256],
                   start=(k == 0), stop=(k == NKC - 1))
            nc.vector.tensor_copy(
                v_r[:, j, :, 0:HD],
                pv[:, 0:HPC * HD].rearrange("p (h e) -> p h e", e=HD))

    # ---- attention block (emitted interleaved with projection chunks) ----
    def attention_block(sqb, hlist):
        nch = (sqb + 1) * CPB
        cps = {h: psB.tile([128, SC], F32, tag="pb", name=f"cps{h}_{sqb}")
               for h in hlist}
        slots = [(h, ck) for ck in range(nch) for h in hlist]
        for g0 in range(0, len(slots), 3):
            grp = slots[g0:g0 + 3]
            sg = psA.tile([128, 3 * SC], F32, tag="ps", name=f"sg{sqb}_{g0}")
            eg = expp.tile([128, 3 * SC], BF16, tag="exp", name=f"eg{sqb}_{g0}")
            for si, (h, ck) in enumerate(grp):
                osl = slice(si * SC, (si + 1) * SC)
                if h < 2:
                    b0 = h * 64
                    MM(sg[:, osl],
                       kT[ck // CPB][b0:b0 + 64, (ck % CPB) * 128:(ck % CPB + 1) * 128],
                       qT[sqb][b0:b0 + 64, :],
                       start=True, stop=True)
                else:
                    MM(sg[:, osl],
                       q2k2[ck // CPB][:, SC + (ck % CPB) * 128: SC + (ck % CPB + 1) * 128],
                       q2k2[sqb][:, 0:SC],
                       start=True, stop=True)
            n = len(grp) * SC
            nc.scalar.activation(eg[:, 0:n], sg[:, 0:n],
                                 mybir.ActivationFunctionType.Exp, scale=0.125)
            for si, (h, ck) in enumerate(grp):
                osl = slice(si * SC, (si + 1) * SC)
                if ck >= nch - CPB:  # diagonal chunk: causal mask via fill
                    o = (ck - (nch - CPB)) * 128
                    # keep where sq_local >= sk_local + o: -part + free - o >= 0
                    nc.gpsimd.affine_select(
                        eg[:, osl], eg[:, osl], pattern=[[1, SC]],
                        compare_op=GE, fill=0.0, base=-o, channel_multiplier=-1)
                v_r = vt[ck // CPB][:].rearrange("p (c h e) -> p c h e", h=HPC, e=HD + 1)
                MM(cps[h][0:HD + 1, :],
                   v_r[:, ck % CPB, h, :],
                   eg[:, osl],
                   start=(ck == 0), stop=(ck == nch - 1))
        for h in hlist:
            # reciprocal_approx_fast is a bitwise-seed op and misreads PSUM;
            # bounce the denominator row through SBUF first
            dn = smp.tile([1, SC], F32, tag="dn", name=f"dn{h}_{sqb}")
            nc.vector.tensor_copy(dn[:], cps[h][HD:HD + 1, :])
            rec = smp.tile([1, SC], F32, tag="rec", name=f"rec{h}_{sqb}")
            nc.vector.reciprocal_approx_fast(rec[:], dn[:])
            bc = smp.tile([64, SC], F32, tag="bc", name=f"bc{h}_{sqb}")
            nc.gpsimd.partition_broadcast(bc[:], rec[:])
            nc.vector.tensor_mul(ctx[h][sqb][:], cps[h][0:HD, :], bc[:])

    # ---- partial output projection for one 128-chunk ----
    def outproj_chunk(sc2):
        po = psA.tile([128, 1024], F32, tag="ps", name=f"po{sc2}")
        csl = slice((sc2 % CPB) * 128, (sc2 % CPB + 1) * 128)
        for noff, nsz in ((0, 512), (512, 256)):
            for hi in range(HPC):
                MM(po[:, noff:noff + nsz],
                   ctx[hi][sc2 // CPB][:, csl],
                   wo3_sb[:, hi * 768 + noff: hi * 768 + noff + nsz],
                   start=(hi == 0), stop=(hi == HPC - 1))
        ost = ostp.tile([128, 768], F32, tag="ost", name=f"ost{sc2}")
        nc.vector.tensor_copy(ost[:], po[:, 0:768])
        nc.sync.dma_start(out_ap[sc2 * 128:(sc2 + 1) * 128, :], ost[:])

    # ---- interleaved emission so projection / attention / output overlap ----
    # attention block sqb consumes only projection chunks 0..sqb; output
    # chunks 4*sqb..4*sqb+3 consume only attention block sqb
    for sc in range(NSC):
        project_chunk(sc)
        attention_block(sc, (0, 1))
        attention_block(sc, (2,))
        for j in range(CPB):
            outproj_chunk(sc * CPB + j)

    for p in (psB, psA, ostp, smp, expp, ctxp, vp, q2p, kTp, qTp, xtp, constp):
        p.release()


def _build():
    if "nc" in _CACHE:
        return _CACHE["nc"]
    nc = bacc.Bacc("TRN2", target_bir_lowering=False, debug=False, num_devices=N_CORES)
    xT = nc.dram_tensor("xT", [D, S], F32R, kind="ExternalInput").ap()
    wqk = nc.dram_tensor("wqk", [D, 384], F32R, kind="ExternalInput").ap()
    wv = nc.dram_tensor("wv", [D, 256], F32R, kind="ExternalInput").ap()
    wo = nc.dram_tensor("wo", [HPC * HD, D], BF16, kind="ExternalInput").ap()
    out = nc.dram_tensor("out", [S, D], F32, kind="ExternalOutput").ap()
    with tile.TileContext(nc) as tc:
        _emit(nc, tc, (xT, wqk, wv, wo), out)
    nc.compile()
    _CACHE["nc"] = nc
    return nc


def _in_maps(x, w_qkv, w_out):
    import ml_dtypes
    xTs = [np.ascontiguousarray(x[b].T) for b in range(B)]
    maps = []
    for c in range(N_CORES):
        b = c // 4
        h0 = (c % 4) * HPC
        cols = lambda base, h: w_qkv[:, base + (h0 + h) * HD: base + (h0 + h + 1) * HD]
        wqk = np.ascontiguousarray(np.concatenate(
            [cols(0, 0), cols(0, 1),            # m0: qA | qB
             cols(0, 2), cols(D, 2),            # m1: qC | kC
             cols(D, 0), cols(D, 1)], axis=1))  # m2: kA | kB
        wv = np.ascontiguousarray(np.concatenate(
            [cols(2 * D, 0), cols(2 * D, 1), cols(2 * D, 2),
             np.zeros((D, 64), np.float32)], axis=1))
        wo = np.ascontiguousarray(
            w_out[h0 * HD:(h0 + HPC) * HD, :]).astype(ml_dtypes.bfloat16)
        maps.append({"xT": xTs[b], "wqk": wqk, "wv": wv, "wo": wo})
    return maps


def run_sharded(x, w_qkv, w_out, **spmd_kwargs):
    nc = _build()
    res = run_bass_kernel_spmd(nc, _in_maps(x, w_qkv, w_out),
                               list(range(N_CORES)), **spmd_kwargs)
    outs = [res.results[c]["out"] for c in range(N_CORES)]
    y = np.empty((B, S, D), np.float32)
    for b in range(B):
        y[b] = outs[4 * b] + outs[4 * b + 1] + outs[4 * b + 2] + outs[4 * b + 3]
    return y, res


def kernel(x, w_qkv, w_out):
    x = np.asarray(x, dtype=np.float32)
    w_qkv = np.asarray(w_qkv, dtype=np.float32)
    w_out = np.asarray(w_out, dtype=np.float32)
    y, _ = run_sharded(x, w_qkv, w_out)
    return y


# revision 21
# speedup vs baseline: 1.0654x; 1.0623x over previous
"""Causal self-attention (B=2, S=4096, D=768, H=12) on 8 Trainium2 NeuronCores.

Sharding: data + head parallel. Core c handles batch c//4 and the 3 heads
starting at (c%4)*3. Each core computes the qkv projection for its heads,
causal attention, and a partial output projection (its heads' rows of w_out);
the host sums the 4 partial outputs per batch.

Device design notes:
 - x arrives pre-transposed (xT [768, 4096]) so the contraction dim lands on
   SBUF partitions for every projection matmul.
 - q, k are produced transposed ([hd, S]); scores are computed transposed
   ([sk, sq]) so the PV matmul consumes exp(scores) directly as the moving
   operand; a ones-column appended to v yields softmax denominators for free
   in the same matmul.
 - All matmuls run in bf16 (fp32 PSUM accumulate).
 - exp runs on ScalarE with the 1/sqrt(hd) scale fused into the activation
   affine; no max subtraction (scores are O(5) here, exp is safe in fp32).
 - Causal masking: only the 4 diagonal-chunk patterns need masking, applied
   as a GPSIMD affine_select (predicated fill) on exp(scores).
 - Softmax division: reciprocal_approx_fast (~51 ULP) on the denominator row,
   GPSIMD partition-broadcast, one DVE multiply.
 - All persistent activations are split into per-512-chunk tiles so the Tile
   scheduler can overlap projection, attention, and output phases.
"""

import numpy as np

try:
    import concourse.bass as bass  # noqa: F401
except ImportError:
    import sys
    sys.path.insert(0, "/opt/trn_rl_repo")

import concourse.bass as bass
import concourse.tile as tile
from concourse import bacc, mybir
from concourse.bass_utils import run_bass_kernel_spmd

F32 = mybir.dt.float32
F32R = mybir.dt.float32r
BF16 = mybir.dt.bfloat16
N_CORES = 8
B, S, D, H, HD = 2, 4096, 768, 12, 64
HPC = 3            # heads per core
SC = 512           # sequence chunk (free dim of most matmuls)
NSC = S // SC      # 8
KC = 128           # contraction chunk
NKC = D // KC      # 6
NQB = S // SC      # query blocks of 512
CPB = SC // KC     # key chunks per query block (4)
VW = HPC * (HD + 1)  # 195 v columns per key-chunk: [64 v | 1] x 3 heads

_CACHE = {}


def _emit(nc, tc, ins, out_ap):
    xT, wqk, wv, wo = ins
    MM = nc.tensor.matmul
    GE = mybir.AluOpType.is_ge

    constp = tc.alloc_tile_pool(name="const", bufs=1)
    xtp = tc.alloc_tile_pool(name="xt", bufs=8)
    qTp = tc.alloc_tile_pool(name="qTp", bufs=8)
    kTp = tc.alloc_tile_pool(name="kTp", bufs=8)
    q2p = tc.alloc_tile_pool(name="q2p", bufs=8)
    vp = tc.alloc_tile_pool(name="vp", bufs=8)
    ctxp = tc.alloc_tile_pool(name="ctx", bufs=24)
    expp = tc.alloc_tile_pool(name="exp", bufs=3)
    smp = tc.alloc_tile_pool(name="sm", bufs=3)
    ostp = tc.alloc_tile_pool(name="ost", bufs=3)
    psA = tc.alloc_tile_pool(name="psA", bufs=2, space="PSUM")
    psB = tc.alloc_tile_pool(name="psB", bufs=2, space="PSUM")

    # ---- constants ----
    wqk_sb = constp.tile([128, NKC * 384], BF16, tag="wqk")
    for k in range(NKC):
        nc.sync.dma_start(wqk_sb[:, k * 384:(k + 1) * 384], wqk[k * 128:(k + 1) * 128, :])
    wv_sb = constp.tile([128, NKC * 256], BF16, tag="wv")
    for k in range(NKC):
        nc.sync.dma_start(wv_sb[:, k * 256:(k + 1) * 256], wv[k * 128:(k + 1) * 128, :])
    wo3_sb = constp.tile([64, HPC * 768], BF16, tag="wo3")
    for h in range(HPC):
        nc.sync.dma_start(wo3_sb[:, h * 768:(h + 1) * 768], wo[h * 64:(h + 1) * 64, :])

    # persistent activations, one tile per 512-wide sequence chunk.
    # matmul needs lhsT/rhs on the same partitions: head A uses rows 0:64 and
    # head B rows 64:128 of qT/kT (row-tiled concurrent matmuls); head C's
    # q^T and k^T sit side by side at rows 0:64 of q2k2 ([.,0:512]=q, 512:1024=k).
    qT = [qTp.tile([128, SC], BF16, tag="qT", name=f"qT{i}") for i in range(NSC)]
    kT = [kTp.tile([128, SC], BF16, tag="kT", name=f"kT{i}") for i in range(NSC)]
    q2k2 = [q2p.tile([64, 2 * SC], BF16, tag="qk2", name=f"q2k2_{i}") for i in range(NSC)]
    vt = [vp.tile([128, CPB * VW], BF16, tag="v", name=f"vt{i}") for i in range(NSC)]
    ctx = {h: [ctxp.tile([64, SC], BF16, tag="ctx", name=f"ctx{h}_{i}")
               for i in range(NQB)] for h in range(HPC)}

    ones_st = smp.tile([128, CPB], F32, tag="ones")
    nc.vector.memset(ones_st[:], 1.0)
    for i in range(NSC):
        v_r = vt[i][:].rearrange("p (c h e) -> p c h e", h=HPC, e=HD + 1)
        for h in range(HPC):
            nc.vector.tensor_copy(v_r[:, :, h, HD], ones_st[:])

    # ---- qkv projection for one 512-chunk ----
    def project_chunk(sc):
        xts = []
        for k in range(NKC):
            xt = xtp.tile([128, SC], BF16, tag="xt", name=f"xt{k}_{sc}")
            nc.sync.dma_start(xt[:], xT[k * 128:(k + 1) * 128, sc * SC:(sc + 1) * SC])
            xts.append(xt)
        # q/k projection, transposed outputs; m0=[qA|qB], m1=[qC|kC], m2=[kA|kB]
        ps = psA.tile([128, 3 * SC], F32, tag="ps", name=f"psqk{sc}")
        for m in range(3):
            for k in range(NKC):
                MM(ps[:, m * SC:(m + 1) * SC],
                   wqk_sb[:, k * 384 + m * 128: k * 384 + (m + 1) * 128],
                   xts[k][:],
                   start=(k == 0), stop=(k == NKC - 1))
        nc.vector.tensor_copy(qT[sc][:], ps[:, 0:SC])
        nc.vector.tensor_copy(kT[sc][:], ps[:, 2 * SC:3 * SC])
        nc.vector.tensor_copy(q2k2[sc][:, 0:SC], ps[0:64, SC:2 * SC])
        # kC sits at psum rows 64:128 but must land at rows 0:64 -> partition
        # shift via SBUF staging + DMA (DVE lanes cannot cross partitions)
        qst = smp.tile([64, SC], BF16, tag="qst", name=f"qst{sc}")
        nc.vector.tensor_copy(qst[:], ps[64:128, SC:2 * SC])
        nc.sync.dma_start(q2k2[sc][:, SC:2 * SC], qst[:])
        # v projection, natural layout, 4 row chunks of 128
        v_r = vt[sc][:].rearrange("p (c h e) -> p c h e", h=HPC, e=HD + 1)
        for j in range(CPB):
            pv = psB.tile([128, 256], F32, tag="pb", name=f"psv{sc}_{j}")
            for k in range(NKC):
                MM(pv[:], xts[k][:, j * 128:(j + 1) * 128],
                   wv_sb[:, k * 256:(k + 1) * 256],
                   start=(k == 0), stop=(k == NKC - 1))
            nc.vector.tensor_copy(
                v_r[:, j, :, 0:HD],
                pv[:, 0:HPC * HD].rearrange("p (h e) -> p h e", e=HD))

    # ---- attention block (emitted interleaved with projection chunks) ----
    def attention_block(sqb, hlist):
        nch = (sqb + 1) * CPB
        cps = {h: psB.tile([128, SC], F32, tag="pb", name=f"cps{h}_{sqb}")
               for h in hlist}
        slots = [(h, ck) for ck in range(nch) for h in hlist]
        for g0 in range(0, len(slots), 3):
            grp = slots[g0:g0 + 3]
            sg = psA.tile([128, 3 * SC], F32, tag="ps", name=f"sg{sqb}_{g0}")
            eg = expp.tile([128, 3 * SC], BF16, tag="exp", name=f"eg{sqb}_{g0}")
            for si, (h, ck) in enumerate(grp):
                osl = slice(si * SC, (si + 1) * SC)
                if h < 2:
                    b0 = h * 64
                    MM(sg[:, osl],
                       kT[ck // CPB][b0:b0 + 64, (ck % CPB) * 128:(ck % CPB + 1) * 128],
                       qT[sqb][b0:b0 + 64, :],
                       start=True, stop=True)
                else:
                    MM(sg[:, osl],
                       q2k2[ck // CPB][:, SC + (ck % CPB) * 128: SC + (ck % CPB + 1) * 128],
                       q2k2[sqb][:, 0:SC],
                       start=True, stop=True)
            n = len(grp) * SC
            nc.scalar.activation(eg[:, 0:n], sg[:, 0:n],
                                 mybir.ActivationFunctionType.Exp, scale=0.125)
            for si, (h, ck) in enumerate(grp):
                osl = slice(si * SC, (si + 1) * SC)
                if ck >= nch - CPB:  # diagonal chunk: causal mask via fill
                    o = (ck - (nch - CPB)) * 128
                    # keep where sq_local >= sk_local + o: -part + free - o >= 0
                    nc.gpsimd.affine_select(
                        eg[:, osl], eg[:, osl], pattern=[[1, SC]],
                        compare_op=GE, fill=0.0, base=-o, channel_multiplier=-1)
                v_r = vt[ck // CPB][:].rearrange("p (c h e) -> p c h e", h=HPC, e=HD + 1)
                MM(cps[h][0:HD + 1, :],
                   v_r[:, ck % CPB, h, :],
                   eg[:, osl],
                   start=(ck == 0), stop=(ck == nch - 1))
        for h in hlist:
            # reciprocal_approx_fast is a bitwise-seed op and misreads PSUM;
            # bounce the denominator row through SBUF first
            dn = smp.tile([1, SC], F32, tag="dn", name=f"dn{h}_{sqb}")
            nc.vector.tensor_copy(dn[:], cps[h][HD:HD + 1, :])
            rec = smp.tile([1, SC], F32, tag="rec", name=f"rec{h}_{sqb}")
            nc.vector.reciprocal_approx_fast(rec[:], dn[:])
            bc = smp.tile([64, SC], F32, tag="bc", name=f"bc{h}_{sqb}")
            nc.gpsimd.partition_broadcast(bc[:], rec[:])
            nc.vector.tensor_mul(ctx[h][sqb][:], cps[h][0:HD, :], bc[:])

    # ---- partial output projection for one 128-chunk ----
    def outproj_chunk(sc2):
        po = psA.tile([128, 1024], F32, tag="ps", name=f"po{sc2}")
        csl = slice((sc2 % CPB) * 128, (sc2 % CPB + 1) * 128)
        for noff, nsz in ((0, 512), (512, 256)):
            for hi in range(HPC):
                MM(po[:, noff:noff + nsz],
                   ctx[hi][sc2 // CPB][:, csl],
                   wo3_sb[:, hi * 768 + noff: hi * 768 + noff + nsz],
                   start=(hi == 0), stop=(hi == HPC - 1))
        ost = ostp.tile([128, 768], F32, tag="ost", name=f"ost{sc2}")
        nc.vector.tensor_copy(ost[:], po[:, 0:768])
        nc.sync.dma_start(out_ap[sc2 * 128:(sc2 + 1) * 128, :], ost[:])

    # ---- interleaved emission so projection / attention / output overlap ----
    # attention block sqb consumes only projection chunks 0..sqb; output
    # chunks 4*sqb..4*sqb+3 consume only attention block sqb
    for sc in range(NSC):
        project_chunk(sc)
        attention_block(sc, (0, 1))
        attention_block(sc, (2,))
        for j in range(CPB):
            outproj_chunk(sc * CPB + j)

    for p in (psB, psA, ostp, smp, expp, ctxp, vp, q2p, kTp, qTp, xtp, constp):
        p.release()


def _build():
    if "nc" in _CACHE:
        return _CACHE["nc"]
    nc = bacc.Bacc("TRN2", target_bir_lowering=False, debug=False, num_devices=N_CORES)
    xT = nc.dram_tensor("xT", [D, S], BF16, kind="ExternalInput").ap()
    wqk = nc.dram_tensor("wqk", [D, 384], BF16, kind="ExternalInput").ap()
    wv = nc.dram_tensor("wv", [D, 256], BF16, kind="ExternalInput").ap()
    wo = nc.dram_tensor("wo", [HPC * HD, D], BF16, kind="ExternalInput").ap()
    out = nc.dram_tensor("out", [S, D], F32, kind="ExternalOutput").ap()
    with tile.TileContext(nc) as tc:
        _emit(nc, tc, (xT, wqk, wv, wo), out)
    nc.compile()
    _CACHE["nc"] = nc
    return nc


def _in_maps(x, w_qkv, w_out):
    import ml_dtypes
    xTs = [np.ascontiguousarray(x[b].T).astype(ml_dtypes.bfloat16) for b in range(B)]
    maps = []
    for c in range(N_CORES):
        b = c // 4
        h0 = (c % 4) * HPC
        cols = lambda base, h: w_qkv[:, base + (h0 + h) * HD: base + (h0 + h + 1) * HD]
        wqk = np.ascontiguousarray(np.concatenate(
            [cols(0, 0), cols(0, 1),            # m0: qA | qB
             cols(0, 2), cols(D, 2),            # m1: qC | kC
             cols(D, 0), cols(D, 1)], axis=1)).astype(ml_dtypes.bfloat16)
        wv = np.ascontiguousarray(np.concatenate(
            [cols(2 * D, 0), cols(2 * D, 1), cols(2 * D, 2),
             np.zeros((D, 64), np.float32)], axis=1)).astype(ml_dtypes.bfloat16)
        wo = np.ascontiguousarray(
            w_out[h0 * HD:(h0 + HPC) * HD, :]).astype(ml_dtypes.bfloat16)
        maps.append({"xT": xTs[b], "wqk": wqk, "wv": wv, "wo": wo})
    return maps


def run_sharded(x, w_qkv, w_out, **spmd_kwargs):
    nc = _build()
    res = run_bass_kernel_spmd(nc, _in_maps(x, w_qkv, w_out),
                               list(range(N_CORES)), **spmd_kwargs)
    outs = [res.results[c]["out"] for c in range(N_CORES)]
    y = np.empty((B, S, D), np.float32)
    for b in range(B):
        y[b] = outs[4 * b] + outs[4 * b + 1] + outs[4 * b + 2] + outs[4 * b + 3]
    return y, res


def kernel(x, w_qkv, w_out):
    x = np.asarray(x, dtype=np.float32)
    w_qkv = np.asarray(w_qkv, dtype=np.float32)
    w_out = np.asarray(w_out, dtype=np.float32)
    y, _ = run_sharded(x, w_qkv, w_out)
    return y
